# revision 21
# baseline (speedup 1.0000x reference)
"""Trainium2 Bass kernel for nn_KnowledgeGraphGNN (8-node complete-graph GCN over a batch).

Math (exact algebra, valid for any inputs):
  w[b,:]  = softmax(latent[b,:8]);  X[b,n,f] = NF[n,f] + 0.1*w[b,n]
  X@W1    = C1[n,h] + 0.1*w[b,n]*s1[h]          (C1 = NF@W1, s1 = colsum W1)
  z[b,i,h]= D1[i,h] + b1[h] + 0.1*U[b,i]*s1[h]  (D1 = A_hat@C1, U = A_hat@w[b])
  out[b,:]= sum_{i,h} relu(z)[b,i,h] * c[i]*W2[h,:] + b2   (c = colmean of A_hat)

Softmax normalization is folded through the ReLU (positive homogeneity):
with E = exp(latent[:, :8]) and S = sum_j E,
  S*z[b,i,h] = sum_j (d1[i,h] + 0.1*s1[h]*A_hat[i,j]) * E[b,j]
so each hidden tile is a K=8 matmul of E^T against a host-folded constant,
then ReLU, then a K=2048 matmul against c[i]*W2, then a 1/S per-row scale
(b2 rides along as an extra K=8 matmul contributing S[b]*b2[o]).

Sharding: pure data-parallel, batch 8192 -> 8 cores x 1024.
"""

import os
import numpy as np

B, NNODE, FDIM, HDIM, ODIM = 8192, 8, 512, 256, 128
NCORES = 8
BC = B // NCORES          # 1024 batch rows per core
NCHUNK = 16               # h-chunks: partitions hold p = i*16 + (h % 16), h = cc*16 + p%16
HALF = 512                # fp32 matmul max moving free dim
KPAD = 32                 # contraction dim for the K=8 matmuls, zero-padded to 32

_CACHE = {}
LAST_RESULTS = None       # BassKernelResults of the most recent run (for profiling)


def _build_nc():
    import concourse.bacc as bacc
    import concourse.mybir as mybir
    from concourse.tile import TileContext
    from concourse._compat import get_trn_type

    fp32 = mybir.dt.float32
    bf16 = mybir.dt.bfloat16
    AF = mybir.ActivationFunctionType

    nc = bacc.Bacc(get_trn_type() or "TRN2", target_bir_lowering=False, debug=True)

    d_latT = nc.dram_tensor("latT", [KPAD, BC], fp32, kind="ExternalInput")
    d_lat8 = nc.dram_tensor("lat8", [BC, NNODE], fp32, kind="ExternalInput")
    d_zlhs = nc.dram_tensor("zlhs", [128, NCHUNK // 4, 128], bf16, kind="ExternalInput")
    d_w2pk = nc.dram_tensor("w2pk", [128, NCHUNK, ODIM], bf16, kind="ExternalInput")
    d_b2r = nc.dram_tensor("b2r", [KPAD, ODIM], bf16, kind="ExternalInput")
    d_out = nc.dram_tensor("out", [BC, ODIM], fp32, kind="ExternalOutput")

    with TileContext(nc) as tc:
        with (
            tc.tile_pool(name="consts", bufs=1) as consts,
            tc.tile_pool(name="work", bufs=1) as work,
            tc.tile_pool(name="hbuf", bufs=1) as hbuf,
            tc.tile_pool(name="outsb", bufs=3) as outsb,
            tc.tile_pool(name="wpsum", bufs=1, space="PSUM") as wpsum,
            tc.tile_pool(name="zpsum", bufs=5, space="PSUM") as zpsum,
            tc.tile_pool(name="opsum", bufs=2, space="PSUM") as opsum,
        ):
            # ---- PE warmup: dense junk matmuls while input DMAs land, to
            # trip the HAM clock gate to 8/8 (2.4 GHz) before real work ----
            wm_lhs = work.tile([KPAD, 32], bf16)
            nc.vector.memset(wm_lhs[:], 0.0)
            wm_rhs = work.tile([KPAD, 256], bf16)
            nc.vector.memset(wm_rhs[:], 0.0)
            wm_ps = wpsum.tile([32, 256], fp32)
            for _ in range(8):
                nc.tensor.matmul(wm_ps[:], wm_lhs[:], wm_rhs[:], start=True, stop=True)

            # ---- inputs (latency-critical first; all on the sync queue in
            # this order so the big const can't delay the small ones) ----
            sb_latT = work.tile([KPAD, BC], fp32)
            nc.sync.dma_start(out=sb_latT[:], in_=d_latT[:])
            # batch-on-partition copy: [128, t, j] with b = t*128 + p
            sb_lat8 = work.tile([128, BC // 128, NNODE], fp32)
            nc.sync.dma_start(
                out=sb_lat8[:], in_=d_lat8.rearrange("(t p) j -> p t j", p=128)
            )
            sb_zlhs = consts.tile([128, NCHUNK // 4, 128], bf16)
            nc.sync.dma_start(out=sb_zlhs[:], in_=d_zlhs[:])
            sb_b2r = consts.tile([KPAD, ODIM], bf16)
            nc.sync.dma_start(out=sb_b2r[:], in_=d_b2r[:])
            sb_w2 = consts.tile([128, NCHUNK, ODIM], bf16)
            nc.sync.dma_start(out=sb_w2[:], in_=d_w2pk[:])

            # ---- softmax pieces ----
            # E^T = exp(latT) once (latT rows 8:32 are -1e30 so exp gives 0 —
            # K padded to 32 because <32-partition contractions fault on HW),
            # then DMA-replicated to partitions 32/64/96 for the four
            # row-groups of the packed Z matmuls.
            sb_ET = work.tile([128, BC], bf16)
            nc.scalar.activation(out=sb_ET[:KPAD, :], in_=sb_latT[:], func=AF.Exp)
            for r in range(1, 4):
                nc.sync.dma_start(out=sb_ET[32 * r : 32 * (r + 1), :], in_=sb_ET[:KPAD, :])
            # E2 = exp(lat8)    [128, t, j] ; S2 = sum_j ; R2 = 1/S2   [128, t]
            sb_E2 = work.tile([128, BC // 128, NNODE], fp32)
            nc.scalar.activation(out=sb_E2[:], in_=sb_lat8[:], func=AF.Exp)
            sb_S2 = work.tile([128, BC // 128], fp32)
            nc.vector.reduce_sum(out=sb_S2[:], in_=sb_E2[:], axis=mybir.AxisListType.X)
            sb_R2 = work.tile([128, BC // 128], fp32)
            nc.vector.reciprocal(out=sb_R2[:], in_=sb_S2[:])

            # ---- hidden: Z_cc = zlhs_cc^T @ E^T -> relu -> H_cc  [128, BC] ----
            # cc = 4g + r lives at row-group r, free column g; four chunks run
            # concurrently in the PE array via tile_position row packing.
            h_tiles = [None] * NCHUNK
            for g in range(NCHUNK // 4):
                for half in range(BC // HALF):
                    for r in range(4):
                        cc = 4 * g + r
                        if h_tiles[cc] is None:
                            h_tiles[cc] = hbuf.tile(
                                [128, BC], bf16, tag=f"h{cc}", name=f"h{cc}"
                            )
                        z_ps = zpsum.tile([128, HALF], fp32)
                        nc.tensor.matmul(
                            z_ps[:],
                            sb_zlhs[32 * r : 32 * (r + 1), g, :],
                            sb_ET[32 * r : 32 * (r + 1), half * HALF : (half + 1) * HALF],
                            start=True,
                            stop=True,
                            tile_position=(32 * r, 0),
                        )
                        dst = h_tiles[cc][:, half * HALF : (half + 1) * HALF]
                        if (cc + half) % 2 == 0:
                            nc.scalar.activation(out=dst, in_=z_ps[:], func=AF.Relu)
                        else:
                            nc.vector.tensor_scalar_max(dst, z_ps[:], 0.0)

            # ---- output: out[bq] = (sum_cc H_cc^T[:, bq128].T @ W2_cc + S*b2) * r ----
            o_all = outsb.tile([128, BC // 128, ODIM], fp32)
            for bq in range(BC // 128):
                o_ps = opsum.tile([128, ODIM], fp32)
                for cc in range(NCHUNK):
                    nc.tensor.matmul(
                        o_ps[:],
                        h_tiles[cc][:, bq * 128 : (bq + 1) * 128],
                        sb_w2[:, cc, :],
                        start=(cc == 0),
                        stop=False,
                    )
                # + S[b] * b2[o]
                nc.tensor.matmul(
                    o_ps[:],
                    sb_ET[:KPAD, bq * 128 : (bq + 1) * 128],
                    sb_b2r[:],
                    start=False,
                    stop=True,
                )
                nc.scalar.activation(
                    out=o_all[:, bq, :], in_=o_ps[:], func=AF.Copy,
                    scale=sb_R2[:, bq : bq + 1],
                )
            nc.sync.dma_start(
                out=d_out.rearrange("(q p) o -> p q o", p=128), in_=o_all[:]
            )

    nc.finalize()
    return nc


def _host_constants(node_features, edge_attr, W1, b1, W2, b2):
    nf = np.asarray(node_features, np.float32)
    ew = np.asarray(edge_attr, np.float32)[:, 0]
    W1 = np.asarray(W1, np.float32)
    b1 = np.asarray(b1, np.float32)
    W2 = np.asarray(W2, np.float32)
    b2 = np.asarray(b2, np.float32)

    # A_hat = D^-1/2 (A + I) D^-1/2, edges (i, j) for all i != j row-major
    src = np.array([i for i in range(NNODE) for j in range(NNODE) if i != j], np.int64)
    dst = np.array([j for i in range(NNODE) for j in range(NNODE) if i != j], np.int64)
    A = np.zeros((NNODE, NNODE), np.float32)
    A[dst, src] = ew
    A = A + np.eye(NNODE, dtype=np.float32)
    deg = A.sum(axis=1)
    dinv = np.where(deg > 0, deg.astype(np.float32) ** -0.5, 0.0).astype(np.float32)
    A_hat = dinv[:, None] * A * dinv[None, :]

    C1 = nf @ W1                      # [8, 256]
    D1 = A_hat @ C1                   # [8, 256]
    d1 = D1 + b1[None, :]             # [8, 256]
    s1 = W1.sum(axis=0)               # [256]
    cvec = A_hat.mean(axis=0)         # [8]

    p = np.arange(128)
    ip = p // 16                      # node index per partition
    qp = p % 16                       # h sub-index per partition

    import ml_dtypes
    bf16 = ml_dtypes.bfloat16

    # zlhs[j, cc, p] = d1[i(p), h(cc,p)] + 0.1*s1[h(cc,p)]*A_hat[i(p), j],
    # packed for 4x row-tiling: chunk cc = 4g + r at partitions 32r+j, column g
    zlhs = np.zeros((128, NCHUNK // 4, 128), np.float32)
    for cc in range(NCHUNK):
        h = cc * 16 + qp              # [128]
        g, r = cc // 4, cc % 4
        zlhs[32 * r : 32 * r + NNODE, g, :] = (
            d1[ip, h][None, :] + 0.1 * s1[h][None, :] * A_hat[ip, :].T
        )

    # w2pk[p, cc, o] = c[i(p)] * W2[h(cc,p), o]
    w2pk = np.empty((128, NCHUNK, ODIM), np.float32)
    for cc in range(NCHUNK):
        h = cc * 16 + qp
        w2pk[:, cc, :] = cvec[ip][:, None] * W2[h, :]

    b2r = np.zeros((KPAD, ODIM), np.float32)
    b2r[:NNODE, :] = b2[None, :]
    return zlhs.astype(bf16), w2pk.astype(bf16), b2r.astype(bf16)


def kernel(latent_vec, node_features, edge_attr, W1, b1, W2, b2):
    global LAST_RESULTS
    from concourse.bass_utils import run_bass_kernel_spmd

    if "nc" not in _CACHE:
        _CACHE["nc"] = _build_nc()
    nc = _CACHE["nc"]

    zlhs, w2pk, b2r = _host_constants(node_features, edge_attr, W1, b1, W2, b2)

    lat8 = np.ascontiguousarray(np.asarray(latent_vec, np.float32)[:, :NNODE])
    in_maps = []
    for c in range(NCORES):
        sl = lat8[c * BC : (c + 1) * BC]
        latT = np.full((KPAD, BC), -1e30, np.float32)  # exp(-1e30) == 0 pad
        latT[:NNODE] = sl.T
        in_maps.append({
            "latT": latT,
            "lat8": np.ascontiguousarray(sl),
            "zlhs": zlhs,
            "w2pk": w2pk,
            "b2r": b2r,
        })

    trace = bool(int(os.environ.get("GNN_TRACE", "0")))
    kwargs = {}
    if trace:
        kwargs["trace"] = True
        kwargs["trace_cores"] = [int(x) for x in os.environ.get("GNN_TRACE_CORES", "0").split(",")]
    res = run_bass_kernel_spmd(nc, in_maps, core_ids=list(range(NCORES)), **kwargs)
    LAST_RESULTS = res

    out = np.concatenate([res.results[c]["out"] for c in range(NCORES)], axis=0)
    return out


# revision 25
# speedup vs baseline: 1.0186x; 1.0186x over previous
"""Trainium2 Bass kernel for nn_KnowledgeGraphGNN (8-node complete-graph GCN over a batch).

Math (exact algebra, valid for any inputs):
  w[b,:]  = softmax(latent[b,:8]);  X[b,n,f] = NF[n,f] + 0.1*w[b,n]
  X@W1    = C1[n,h] + 0.1*w[b,n]*s1[h]          (C1 = NF@W1, s1 = colsum W1)
  z[b,i,h]= D1[i,h] + b1[h] + 0.1*U[b,i]*s1[h]  (D1 = A_hat@C1, U = A_hat@w[b])
  out[b,:]= sum_{i,h} relu(z)[b,i,h] * c[i]*W2[h,:] + b2   (c = colmean of A_hat)

Softmax normalization is folded through the ReLU (positive homogeneity):
with E = exp(latent[:, :8]) and S = sum_j E,
  S*z[b,i,h] = sum_j (d1[i,h] + 0.1*s1[h]*A_hat[i,j]) * E[b,j]
so each hidden tile is a K=8 matmul of E^T against a host-folded constant,
then ReLU, then a K=2048 matmul against c[i]*W2, then a 1/S per-row scale
(b2 rides along as an extra K=8 matmul contributing S[b]*b2[o]).

Sharding: pure data-parallel, batch 8192 -> 8 cores x 1024.
"""

import os
import numpy as np

B, NNODE, FDIM, HDIM, ODIM = 8192, 8, 512, 256, 128
NCORES = 8
BC = B // NCORES          # 1024 batch rows per core
NCHUNK = 16               # h-chunks: partitions hold p = i*16 + (h % 16), h = cc*16 + p%16
HALF = 512                # fp32 matmul max moving free dim
KPAD = 32                 # contraction dim for the K=8 matmuls, zero-padded to 32

_CACHE = {}
LAST_RESULTS = None       # BassKernelResults of the most recent run (for profiling)


def _build_nc():
    import concourse.bacc as bacc
    import concourse.mybir as mybir
    from concourse.tile import TileContext
    from concourse._compat import get_trn_type

    fp32 = mybir.dt.float32
    bf16 = mybir.dt.bfloat16
    AF = mybir.ActivationFunctionType

    nc = bacc.Bacc(get_trn_type() or "TRN2", target_bir_lowering=False, debug=True)

    d_latT = nc.dram_tensor("latT", [NNODE, BC], fp32, kind="ExternalInput")
    d_lat8 = nc.dram_tensor("lat8", [BC, NNODE], fp32, kind="ExternalInput")
    d_zlhs = nc.dram_tensor("zlhs", [128, NCHUNK // 4, 128], bf16, kind="ExternalInput")
    d_w2pk = nc.dram_tensor("w2pk", [128, NCHUNK, ODIM], bf16, kind="ExternalInput")
    d_b2r = nc.dram_tensor("b2r", [KPAD, ODIM], bf16, kind="ExternalInput")
    d_out = nc.dram_tensor("out", [BC, ODIM], fp32, kind="ExternalOutput")

    with TileContext(nc) as tc:
        with (
            tc.tile_pool(name="consts", bufs=1) as consts,
            tc.tile_pool(name="work", bufs=1) as work,
            tc.tile_pool(name="hbuf", bufs=1) as hbuf,
            tc.tile_pool(name="outsb", bufs=3) as outsb,
            tc.tile_pool(name="wpsum", bufs=1, space="PSUM") as wpsum,
            tc.tile_pool(name="zpsum", bufs=5, space="PSUM") as zpsum,
            tc.tile_pool(name="opsum", bufs=2, space="PSUM") as opsum,
        ):
            # ---- PE warmup: dense junk matmuls while input DMAs land, to
            # trip the HAM clock gate to 8/8 (2.4 GHz) before real work ----
            # memsets first: ET zero-fill (K padded to 32 per row-group —
            # <32-partition contractions fault on HW) and warmup operands
            sb_ET = work.tile([128, BC], bf16)
            nc.vector.memset(sb_ET[:], 0.0)
            wm_lhs = work.tile([KPAD, 32], bf16)
            nc.vector.memset(wm_lhs[:], 0.0)
            wm_rhs = work.tile([KPAD, 256], bf16)
            nc.vector.memset(wm_rhs[:], 0.0)
            # preload the ACT exp table while DMAs land
            wm_exp = work.tile([KPAD, 1], bf16)
            nc.scalar.activation(out=wm_exp[:], in_=wm_lhs[:, 0:1], func=AF.Exp)
            # dense junk matmuls to trip the HAM clock gate to 8/8 (2.4 GHz)
            wm_ps = wpsum.tile([32, 256], fp32)
            for _ in range(8):
                nc.tensor.matmul(wm_ps[:], wm_lhs[:], wm_rhs[:], start=True, stop=True)

            # ---- inputs: latency-critical on the sync queue, big consts on
            # the gpsimd (SWDGE) queue so they can't delay the small ones ----
            sb_latT = work.tile([NNODE, BC], fp32)
            nc.sync.dma_start(out=sb_latT[:], in_=d_latT[:])
            # batch-on-partition copy: [128, t, j] with b = t*128 + p
            sb_lat8 = work.tile([128, BC // 128, NNODE], fp32)
            nc.sync.dma_start(
                out=sb_lat8[:], in_=d_lat8.rearrange("(t p) j -> p t j", p=128)
            )
            sb_zlhs = consts.tile([128, NCHUNK // 4, 128], bf16)
            nc.gpsimd.dma_start(out=sb_zlhs[:], in_=d_zlhs[:])
            sb_b2r = consts.tile([KPAD, ODIM], bf16)
            nc.gpsimd.dma_start(out=sb_b2r[:], in_=d_b2r[:])
            sb_w2 = consts.tile([128, NCHUNK, ODIM], bf16)
            nc.gpsimd.dma_start(out=sb_w2[:], in_=d_w2pk[:])

            # ---- softmax pieces ----
            # E^T = exp(latT) into rows 0:8, then DMA-replicated to partition
            # groups 32/64/96 for the four row-groups of the packed Z matmuls.
            nc.scalar.activation(out=sb_ET[:NNODE, :], in_=sb_latT[:], func=AF.Exp)
            for r in range(1, 4):
                nc.sync.dma_start(
                    out=sb_ET[32 * r : 32 * r + NNODE, :], in_=sb_ET[:NNODE, :]
                )
            # E2 = exp(lat8)    [128, t, j] ; S2 = sum_j ; R2 = 1/S2   [128, t]
            sb_E2 = work.tile([128, BC // 128, NNODE], fp32)
            nc.scalar.activation(out=sb_E2[:], in_=sb_lat8[:], func=AF.Exp)
            sb_S2 = work.tile([128, BC // 128], fp32)
            nc.vector.reduce_sum(out=sb_S2[:], in_=sb_E2[:], axis=mybir.AxisListType.X)
            sb_R2 = work.tile([128, BC // 128], fp32)
            nc.vector.reciprocal(out=sb_R2[:], in_=sb_S2[:])

            # ---- hidden: Z_cc = zlhs_cc^T @ E^T -> relu -> H_cc  [128, BC] ----
            # cc = 4g + r lives at row-group r, free column g; four chunks run
            # concurrently in the PE array via tile_position row packing.
            h_tiles = [None] * NCHUNK
            for g in range(NCHUNK // 4):
                for half in range(BC // HALF):
                    for r in range(4):
                        cc = 4 * g + r
                        if h_tiles[cc] is None:
                            h_tiles[cc] = hbuf.tile(
                                [128, BC], bf16, tag=f"h{cc}", name=f"h{cc}"
                            )
                        z_ps = zpsum.tile([128, HALF], fp32)
                        nc.tensor.matmul(
                            z_ps[:],
                            sb_zlhs[32 * r : 32 * (r + 1), g, :],
                            sb_ET[32 * r : 32 * (r + 1), half * HALF : (half + 1) * HALF],
                            start=True,
                            stop=True,
                            tile_position=(32 * r, 0),
                        )
                        dst = h_tiles[cc][:, half * HALF : (half + 1) * HALF]
                        if (cc + half) % 2 == 0:
                            nc.scalar.activation(out=dst, in_=z_ps[:], func=AF.Relu)
                        else:
                            nc.vector.tensor_scalar_max(dst, z_ps[:], 0.0)

            # ---- output: out[bq] = (sum_cc H_cc^T[:, bq128].T @ W2_cc + S*b2) * r ----
            o_all = outsb.tile([128, BC // 128, ODIM], fp32)
            for bq in range(BC // 128):
                o_ps = opsum.tile([128, ODIM], fp32)
                for cc in range(NCHUNK):
                    nc.tensor.matmul(
                        o_ps[:],
                        h_tiles[cc][:, bq * 128 : (bq + 1) * 128],
                        sb_w2[:, cc, :],
                        start=(cc == 0),
                        stop=False,
                    )
                # + S[b] * b2[o]
                nc.tensor.matmul(
                    o_ps[:],
                    sb_ET[:KPAD, bq * 128 : (bq + 1) * 128],
                    sb_b2r[:],
                    start=False,
                    stop=True,
                )
                nc.vector.tensor_scalar_mul(
                    o_all[:, bq, :], o_ps[:], sb_R2[:, bq : bq + 1]
                )
            nc.sync.dma_start(
                out=d_out.rearrange("(q p) o -> p q o", p=128), in_=o_all[:]
            )

    nc.finalize()
    return nc


def _host_constants(node_features, edge_attr, W1, b1, W2, b2):
    nf = np.asarray(node_features, np.float32)
    ew = np.asarray(edge_attr, np.float32)[:, 0]
    W1 = np.asarray(W1, np.float32)
    b1 = np.asarray(b1, np.float32)
    W2 = np.asarray(W2, np.float32)
    b2 = np.asarray(b2, np.float32)

    # A_hat = D^-1/2 (A + I) D^-1/2, edges (i, j) for all i != j row-major
    src = np.array([i for i in range(NNODE) for j in range(NNODE) if i != j], np.int64)
    dst = np.array([j for i in range(NNODE) for j in range(NNODE) if i != j], np.int64)
    A = np.zeros((NNODE, NNODE), np.float32)
    A[dst, src] = ew
    A = A + np.eye(NNODE, dtype=np.float32)
    deg = A.sum(axis=1)
    dinv = np.where(deg > 0, deg.astype(np.float32) ** -0.5, 0.0).astype(np.float32)
    A_hat = dinv[:, None] * A * dinv[None, :]

    C1 = nf @ W1                      # [8, 256]
    D1 = A_hat @ C1                   # [8, 256]
    d1 = D1 + b1[None, :]             # [8, 256]
    s1 = W1.sum(axis=0)               # [256]
    cvec = A_hat.mean(axis=0)         # [8]

    p = np.arange(128)
    ip = p // 16                      # node index per partition
    qp = p % 16                       # h sub-index per partition

    import ml_dtypes
    bf16 = ml_dtypes.bfloat16

    # zlhs[j, cc, p] = d1[i(p), h(cc,p)] + 0.1*s1[h(cc,p)]*A_hat[i(p), j],
    # packed for 4x row-tiling: chunk cc = 4g + r at partitions 32r+j, column g
    zlhs = np.zeros((128, NCHUNK // 4, 128), np.float32)
    for cc in range(NCHUNK):
        h = cc * 16 + qp              # [128]
        g, r = cc // 4, cc % 4
        zlhs[32 * r : 32 * r + NNODE, g, :] = (
            d1[ip, h][None, :] + 0.1 * s1[h][None, :] * A_hat[ip, :].T
        )

    # w2pk[p, cc, o] = c[i(p)] * W2[h(cc,p), o]
    w2pk = np.empty((128, NCHUNK, ODIM), np.float32)
    for cc in range(NCHUNK):
        h = cc * 16 + qp
        w2pk[:, cc, :] = cvec[ip][:, None] * W2[h, :]

    b2r = np.zeros((KPAD, ODIM), np.float32)
    b2r[:NNODE, :] = b2[None, :]
    return zlhs.astype(bf16), w2pk.astype(bf16), b2r.astype(bf16)


def kernel(latent_vec, node_features, edge_attr, W1, b1, W2, b2):
    global LAST_RESULTS
    from concourse.bass_utils import run_bass_kernel_spmd

    if "nc" not in _CACHE:
        _CACHE["nc"] = _build_nc()
    nc = _CACHE["nc"]

    zlhs, w2pk, b2r = _host_constants(node_features, edge_attr, W1, b1, W2, b2)

    lat8 = np.ascontiguousarray(np.asarray(latent_vec, np.float32)[:, :NNODE])
    in_maps = []
    for c in range(NCORES):
        sl = lat8[c * BC : (c + 1) * BC]
        in_maps.append({
            "latT": np.ascontiguousarray(sl.T),
            "lat8": np.ascontiguousarray(sl),
            "zlhs": zlhs,
            "w2pk": w2pk,
            "b2r": b2r,
        })

    trace = bool(int(os.environ.get("GNN_TRACE", "0")))
    kwargs = {}
    if trace:
        kwargs["trace"] = True
        kwargs["trace_cores"] = [int(x) for x in os.environ.get("GNN_TRACE_CORES", "0").split(",")]
    res = run_bass_kernel_spmd(nc, in_maps, core_ids=list(range(NCORES)), **kwargs)
    LAST_RESULTS = res

    out = np.concatenate([res.results[c]["out"] for c in range(NCORES)], axis=0)
    return out


# revision 28
# speedup vs baseline: 1.0623x; 1.0430x over previous
"""Trainium2 Bass kernel for nn_KnowledgeGraphGNN (8-node complete-graph GCN over a batch).

Math (exact algebra, valid for any inputs):
  w[b,:]  = softmax(latent[b,:8]);  X[b,n,f] = NF[n,f] + 0.1*w[b,n]
  X@W1    = C1[n,h] + 0.1*w[b,n]*s1[h]          (C1 = NF@W1, s1 = colsum W1)
  z[b,i,h]= D1[i,h] + b1[h] + 0.1*U[b,i]*s1[h]  (D1 = A_hat@C1, U = A_hat@w[b])
  out[b,:]= sum_{i,h} relu(z)[b,i,h] * c[i]*W2[h,:] + b2   (c = colmean of A_hat)

Softmax normalization is folded through the ReLU (positive homogeneity):
with E = exp(latent[:, :8]) and S = sum_j E,
  S*z[b,i,h] = sum_j (d1[i,h] + 0.1*s1[h]*A_hat[i,j]) * E[b,j]
so each hidden tile is a K=8 matmul of E^T against a host-folded constant,
then ReLU, then a K=2048 matmul against c[i]*W2, then a 1/S per-row scale
(b2 rides along as an extra K=8 matmul contributing S[b]*b2[o]).

Sharding: pure data-parallel, batch 8192 -> 8 cores x 1024.
"""

import os
import numpy as np

B, NNODE, FDIM, HDIM, ODIM = 8192, 8, 512, 256, 128
NCORES = 8
BC = B // NCORES          # 1024 batch rows per core
NCHUNK = 16               # h-chunks: partitions hold p = i*16 + (h % 16), h = cc*16 + p%16
HALF = 512                # fp32 matmul max moving free dim
KPAD = 32                 # contraction dim for the K=8 matmuls, zero-padded to 32

_CACHE = {}
LAST_RESULTS = None       # BassKernelResults of the most recent run (for profiling)


def _build_nc():
    import concourse.bacc as bacc
    import concourse.mybir as mybir
    from concourse.tile import TileContext
    from concourse._compat import get_trn_type

    fp32 = mybir.dt.float32
    bf16 = mybir.dt.bfloat16
    AF = mybir.ActivationFunctionType

    nc = bacc.Bacc(get_trn_type() or "TRN2", target_bir_lowering=False, debug=True)

    d_latT = nc.dram_tensor("latT", [NNODE, BC], fp32, kind="ExternalInput")
    d_lat8 = nc.dram_tensor("lat8", [BC, NNODE], fp32, kind="ExternalInput")
    d_zlhs = nc.dram_tensor("zlhs", [128, NCHUNK // 4, 128], bf16, kind="ExternalInput")
    d_w2pk = nc.dram_tensor("w2pk", [128, NCHUNK, ODIM], bf16, kind="ExternalInput")
    d_b2r = nc.dram_tensor("b2r", [KPAD, ODIM], bf16, kind="ExternalInput")
    d_out = nc.dram_tensor("out", [BC, ODIM], fp32, kind="ExternalOutput")

    with TileContext(nc) as tc:
        with (
            tc.tile_pool(name="consts", bufs=1) as consts,
            tc.tile_pool(name="work", bufs=1) as work,
            tc.tile_pool(name="hbuf", bufs=1) as hbuf,
            tc.tile_pool(name="outsb", bufs=3) as outsb,
            tc.tile_pool(name="wpsum", bufs=1, space="PSUM") as wpsum,
            tc.tile_pool(name="zpsum", bufs=5, space="PSUM") as zpsum,
            tc.tile_pool(name="opsum", bufs=2, space="PSUM") as opsum,
        ):
            # ---- PE warmup: dense junk matmuls while input DMAs land, to
            # trip the HAM clock gate to 8/8 (2.4 GHz) before real work ----
            # warmup operand memsets first (tiny), then the big ET zero-fill
            # (K padded to 32 per row-group — <32-partition contractions
            # fault on HW)
            wm_lhs = work.tile([KPAD, 32], bf16)
            nc.vector.memset(wm_lhs[:], 0.0)
            wm_rhs = work.tile([KPAD, 256], bf16)
            nc.vector.memset(wm_rhs[:], 0.0)
            # preload the ACT exp table while DMAs land
            wm_exp = work.tile([KPAD, 1], bf16)
            nc.scalar.activation(out=wm_exp[:], in_=wm_lhs[:, 0:1], func=AF.Exp)
            sb_ET = work.tile([128, BC], bf16)
            nc.vector.memset(sb_ET[:], 0.0)
            # dense junk matmuls to trip the HAM clock gate to 8/8 (2.4 GHz),
            # sized to bridge until the first real Z matmuls are ready
            wm_ps = wpsum.tile([32, 256], fp32)
            for _ in range(12):
                nc.tensor.matmul(wm_ps[:], wm_lhs[:], wm_rhs[:], start=True, stop=True)

            # ---- inputs: latency-critical on the sync queue, big consts on
            # the gpsimd (SWDGE) queue so they can't delay the small ones ----
            sb_latT = work.tile([NNODE, BC], fp32)
            nc.sync.dma_start(out=sb_latT[:], in_=d_latT[:])
            # batch-on-partition copy: [128, t, j] with b = t*128 + p
            sb_lat8 = work.tile([128, BC // 128, NNODE], fp32)
            nc.sync.dma_start(
                out=sb_lat8[:], in_=d_lat8.rearrange("(t p) j -> p t j", p=128)
            )
            sb_zlhs = consts.tile([128, NCHUNK // 4, 128], bf16)
            nc.gpsimd.dma_start(out=sb_zlhs[:], in_=d_zlhs[:])
            sb_b2r = consts.tile([KPAD, ODIM], bf16)
            nc.gpsimd.dma_start(out=sb_b2r[:], in_=d_b2r[:])
            sb_w2 = consts.tile([128, NCHUNK, ODIM], bf16)
            nc.gpsimd.dma_start(out=sb_w2[:], in_=d_w2pk[:])

            # ---- softmax pieces ----
            # E^T = exp(latT) into rows 0:8, then DMA-replicated to partition
            # groups 32/64/96 for the four row-groups of the packed Z matmuls.
            nc.scalar.activation(out=sb_ET[:NNODE, :], in_=sb_latT[:], func=AF.Exp)
            # replicate concurrently: issue from three different engines'
            # DGE queues so the copies don't serialize on one queue
            for r, eng in ((1, nc.sync), (2, nc.scalar), (3, nc.gpsimd)):
                eng.dma_start(
                    out=sb_ET[32 * r : 32 * r + NNODE, :], in_=sb_ET[:NNODE, :]
                )
            # E2 = exp(lat8)    [128, t, j] ; S2 = sum_j ; R2 = 1/S2   [128, t]
            sb_E2 = work.tile([128, BC // 128, NNODE], fp32)
            nc.scalar.activation(out=sb_E2[:], in_=sb_lat8[:], func=AF.Exp)
            sb_S2 = work.tile([128, BC // 128], fp32)
            nc.vector.reduce_sum(out=sb_S2[:], in_=sb_E2[:], axis=mybir.AxisListType.X)
            sb_R2 = work.tile([128, BC // 128], fp32)
            nc.vector.reciprocal(out=sb_R2[:], in_=sb_S2[:])

            # ---- hidden: Z_cc = zlhs_cc^T @ E^T -> relu -> H_cc  [128, BC] ----
            # cc = 4g + r lives at row-group r, free column g; four chunks run
            # concurrently in the PE array via tile_position row packing.
            h_tiles = [None] * NCHUNK
            for g in range(NCHUNK // 4):
                for half in range(BC // HALF):
                    for r in range(4):
                        cc = 4 * g + r
                        if h_tiles[cc] is None:
                            h_tiles[cc] = hbuf.tile(
                                [128, BC], bf16, tag=f"h{cc}", name=f"h{cc}"
                            )
                        z_ps = zpsum.tile([128, HALF], fp32)
                        nc.tensor.matmul(
                            z_ps[:],
                            sb_zlhs[32 * r : 32 * (r + 1), g, :],
                            sb_ET[32 * r : 32 * (r + 1), half * HALF : (half + 1) * HALF],
                            start=True,
                            stop=True,
                            tile_position=(32 * r, 0),
                        )
                        dst = h_tiles[cc][:, half * HALF : (half + 1) * HALF]
                        if (cc + half) % 2 == 0:
                            nc.scalar.activation(out=dst, in_=z_ps[:], func=AF.Relu)
                        else:
                            nc.vector.tensor_scalar_max(dst, z_ps[:], 0.0)

            # ---- output: out[bq] = (sum_cc H_cc^T[:, bq128].T @ W2_cc + S*b2) * r ----
            o_all = outsb.tile([128, BC // 128, ODIM], fp32)
            for bq in range(BC // 128):
                o_ps = opsum.tile([128, ODIM], fp32)
                for cc in range(NCHUNK):
                    nc.tensor.matmul(
                        o_ps[:],
                        h_tiles[cc][:, bq * 128 : (bq + 1) * 128],
                        sb_w2[:, cc, :],
                        start=(cc == 0),
                        stop=False,
                    )
                # + S[b] * b2[o]
                nc.tensor.matmul(
                    o_ps[:],
                    sb_ET[:KPAD, bq * 128 : (bq + 1) * 128],
                    sb_b2r[:],
                    start=False,
                    stop=True,
                )
                nc.vector.tensor_scalar_mul(
                    o_all[:, bq, :], o_ps[:], sb_R2[:, bq : bq + 1]
                )
            nc.sync.dma_start(
                out=d_out.rearrange("(q p) o -> p q o", p=128), in_=o_all[:]
            )

    nc.finalize()
    return nc


def _host_constants(node_features, edge_attr, W1, b1, W2, b2):
    nf = np.asarray(node_features, np.float32)
    ew = np.asarray(edge_attr, np.float32)[:, 0]
    W1 = np.asarray(W1, np.float32)
    b1 = np.asarray(b1, np.float32)
    W2 = np.asarray(W2, np.float32)
    b2 = np.asarray(b2, np.float32)

    # A_hat = D^-1/2 (A + I) D^-1/2, edges (i, j) for all i != j row-major
    src = np.array([i for i in range(NNODE) for j in range(NNODE) if i != j], np.int64)
    dst = np.array([j for i in range(NNODE) for j in range(NNODE) if i != j], np.int64)
    A = np.zeros((NNODE, NNODE), np.float32)
    A[dst, src] = ew
    A = A + np.eye(NNODE, dtype=np.float32)
    deg = A.sum(axis=1)
    dinv = np.where(deg > 0, deg.astype(np.float32) ** -0.5, 0.0).astype(np.float32)
    A_hat = dinv[:, None] * A * dinv[None, :]

    C1 = nf @ W1                      # [8, 256]
    D1 = A_hat @ C1                   # [8, 256]
    d1 = D1 + b1[None, :]             # [8, 256]
    s1 = W1.sum(axis=0)               # [256]
    cvec = A_hat.mean(axis=0)         # [8]

    p = np.arange(128)
    ip = p // 16                      # node index per partition
    qp = p % 16                       # h sub-index per partition

    import ml_dtypes
    bf16 = ml_dtypes.bfloat16

    # zlhs[j, cc, p] = d1[i(p), h(cc,p)] + 0.1*s1[h(cc,p)]*A_hat[i(p), j],
    # packed for 4x row-tiling: chunk cc = 4g + r at partitions 32r+j, column g
    zlhs = np.zeros((128, NCHUNK // 4, 128), np.float32)
    for cc in range(NCHUNK):
        h = cc * 16 + qp              # [128]
        g, r = cc // 4, cc % 4
        zlhs[32 * r : 32 * r + NNODE, g, :] = (
            d1[ip, h][None, :] + 0.1 * s1[h][None, :] * A_hat[ip, :].T
        )

    # w2pk[p, cc, o] = c[i(p)] * W2[h(cc,p), o]
    w2pk = np.empty((128, NCHUNK, ODIM), np.float32)
    for cc in range(NCHUNK):
        h = cc * 16 + qp
        w2pk[:, cc, :] = cvec[ip][:, None] * W2[h, :]

    b2r = np.zeros((KPAD, ODIM), np.float32)
    b2r[:NNODE, :] = b2[None, :]
    return zlhs.astype(bf16), w2pk.astype(bf16), b2r.astype(bf16)


def kernel(latent_vec, node_features, edge_attr, W1, b1, W2, b2):
    global LAST_RESULTS
    from concourse.bass_utils import run_bass_kernel_spmd

    if "nc" not in _CACHE:
        _CACHE["nc"] = _build_nc()
    nc = _CACHE["nc"]

    zlhs, w2pk, b2r = _host_constants(node_features, edge_attr, W1, b1, W2, b2)

    lat8 = np.ascontiguousarray(np.asarray(latent_vec, np.float32)[:, :NNODE])
    in_maps = []
    for c in range(NCORES):
        sl = lat8[c * BC : (c + 1) * BC]
        in_maps.append({
            "latT": np.ascontiguousarray(sl.T),
            "lat8": np.ascontiguousarray(sl),
            "zlhs": zlhs,
            "w2pk": w2pk,
            "b2r": b2r,
        })

    trace = bool(int(os.environ.get("GNN_TRACE", "0")))
    kwargs = {}
    if trace:
        kwargs["trace"] = True
        kwargs["trace_cores"] = [int(x) for x in os.environ.get("GNN_TRACE_CORES", "0").split(",")]
    res = run_bass_kernel_spmd(nc, in_maps, core_ids=list(range(NCORES)), **kwargs)
    LAST_RESULTS = res

    out = np.concatenate([res.results[c]["out"] for c in range(NCORES)], axis=0)
    return out


# revision 33
# speedup vs baseline: 2.1388x; 2.0133x over previous
"""Trainium2 Bass kernel for nn_KnowledgeGraphGNN (8-node complete-graph GCN over a batch).

Math (exact algebra, valid for any inputs):
  w[b,:]  = softmax(latent[b,:8]);  X[b,n,f] = NF[n,f] + 0.1*w[b,n]
  X@W1    = C1[n,h] + 0.1*w[b,n]*s1[h]          (C1 = NF@W1, s1 = colsum W1)
  z[b,i,h]= D1[i,h] + b1[h] + 0.1*U[b,i]*s1[h]  (D1 = A_hat@C1, U = A_hat@w[b])
  out[b,:]= sum_{i,h} relu(z)[b,i,h] * c[i]*W2[h,:] + b2   (c = colmean of A_hat)

Softmax normalization is folded through the ReLU (positive homogeneity):
with E = exp(latent[:, :8]) and S = sum_j E,
  S*z[b,i,h] = sum_j (d1[i,h] + 0.1*s1[h]*A_hat[i,j]) * E[b,j]
so each hidden tile is a K=8 matmul of E^T against a host-folded constant,
then ReLU, then a K=2048 matmul against c[i]*W2, then a 1/S per-row scale
(b2 rides along as an extra K=8 matmul contributing S[b]*b2[o]).

Sharding: pure data-parallel, batch 8192 -> 8 cores x 1024.
"""

import os
import numpy as np

B, NNODE, FDIM, HDIM, ODIM = 8192, 8, 512, 256, 128
NCORES = 8
BC = B // NCORES          # 1024 batch rows per core
NCHUNK = 16               # h-chunks: partitions hold p = i*16 + (h % 16), h = cc*16 + p%16
HALF = 512                # fp32 matmul max moving free dim
KPAD = 32                 # contraction dim for the K=8 matmuls, zero-padded to 32

_CACHE = {}
LAST_RESULTS = None       # BassKernelResults of the most recent run (for profiling)


def _build_nc():
    import concourse.bacc as bacc
    import concourse.mybir as mybir
    from concourse.tile import TileContext
    from concourse._compat import get_trn_type

    fp32 = mybir.dt.float32
    bf16 = mybir.dt.bfloat16
    AF = mybir.ActivationFunctionType

    nc = bacc.Bacc(get_trn_type() or "TRN2", target_bir_lowering=False, debug=True)

    d_latT = nc.dram_tensor("latT", [NNODE, BC], fp32, kind="ExternalInput")
    d_lat8 = nc.dram_tensor("lat8", [BC, NNODE], fp32, kind="ExternalInput")
    d_zlhs = nc.dram_tensor("zlhs", [128, NCHUNK // 4, 128], bf16, kind="ExternalInput")
    d_w2pk = nc.dram_tensor("w2pk", [128, NCHUNK, ODIM], bf16, kind="ExternalInput")
    d_b2r = nc.dram_tensor("b2r", [KPAD, ODIM], bf16, kind="ExternalInput")
    d_out = nc.dram_tensor("out", [BC, ODIM], fp32, kind="ExternalOutput")

    with TileContext(nc) as tc:
        with (
            tc.tile_pool(name="consts", bufs=1) as consts,
            tc.tile_pool(name="work", bufs=1) as work,
            tc.tile_pool(name="hbuf", bufs=1) as hbuf,
            tc.tile_pool(name="outsb", bufs=3) as outsb,
            tc.tile_pool(name="wpsum", bufs=1, space="PSUM") as wpsum,
            tc.tile_pool(name="zpsum", bufs=5, space="PSUM") as zpsum,
            tc.tile_pool(name="opsum", bufs=2, space="PSUM") as opsum,
        ):
            # ---- PE warmup: dense junk matmuls while input DMAs land, to
            # trip the HAM clock gate to 8/8 (2.4 GHz) before real work ----
            # warmup operand memsets first (tiny), then the big ET zero-fill
            # (K padded to 32 per row-group — <32-partition contractions
            # fault on HW)
            wm_lhs = work.tile([KPAD, 32], bf16)
            nc.vector.memset(wm_lhs[:], 0.0)
            wm_rhs = work.tile([KPAD, 256], bf16)
            nc.vector.memset(wm_rhs[:], 0.0)
            # preload the ACT exp table while DMAs land
            wm_exp = work.tile([KPAD, 1], bf16)
            nc.scalar.activation(out=wm_exp[:], in_=wm_lhs[:, 0:1], func=AF.Exp)
            sb_ET = work.tile([128, BC], bf16)
            nc.vector.memset(sb_ET[:], 0.0)
            # dense junk matmuls to trip the HAM clock gate to 8/8 (2.4 GHz),
            # sized to bridge until the first real Z matmuls are ready
            wm_ps = wpsum.tile([32, 256], fp32)
            for _ in range(12):
                nc.tensor.matmul(wm_ps[:], wm_lhs[:], wm_rhs[:], start=True, stop=True)

            # ---- inputs: latency-critical on the sync queue, big consts on
            # the gpsimd (SWDGE) queue so they can't delay the small ones ----
            sb_latT = work.tile([NNODE, BC], fp32)
            nc.sync.dma_start(out=sb_latT[:], in_=d_latT[:])
            # batch-on-partition copy: [128, t, j] with b = t*128 + p
            sb_lat8 = work.tile([128, BC // 128, NNODE], fp32)
            nc.sync.dma_start(
                out=sb_lat8[:], in_=d_lat8.rearrange("(t p) j -> p t j", p=128)
            )
            sb_zlhs = consts.tile([128, NCHUNK // 4, 128], bf16)
            nc.gpsimd.dma_start(out=sb_zlhs[:], in_=d_zlhs[:])
            sb_b2r = consts.tile([KPAD, ODIM], bf16)
            nc.gpsimd.dma_start(out=sb_b2r[:], in_=d_b2r[:])
            sb_w2 = consts.tile([128, NCHUNK, ODIM], bf16)
            nc.gpsimd.dma_start(out=sb_w2[:], in_=d_w2pk[:])

            # ---- softmax pieces ----
            # E^T = exp(latT) into rows 0:8, then DMA-replicated to partition
            # groups 32/64/96 for the four row-groups of the packed Z matmuls.
            nc.scalar.activation(out=sb_ET[:NNODE, :], in_=sb_latT[:], func=AF.Exp)
            # replicate concurrently: issue from three different engines'
            # DGE queues so the copies don't serialize on one queue
            for r, eng in ((1, nc.sync), (2, nc.scalar), (3, nc.gpsimd)):
                eng.dma_start(
                    out=sb_ET[32 * r : 32 * r + NNODE, :], in_=sb_ET[:NNODE, :]
                )
            # E2 = exp(lat8)    [128, t, j] ; S2 = sum_j ; R2 = 1/S2   [128, t]
            sb_E2 = work.tile([128, BC // 128, NNODE], fp32)
            nc.scalar.activation(out=sb_E2[:], in_=sb_lat8[:], func=AF.Exp)
            sb_S2 = work.tile([128, BC // 128], fp32)
            nc.vector.reduce_sum(out=sb_S2[:], in_=sb_E2[:], axis=mybir.AxisListType.X)
            sb_R2 = work.tile([128, BC // 128], fp32)
            nc.vector.reciprocal(out=sb_R2[:], in_=sb_S2[:])

            # ---- hidden: Z_cc = zlhs_cc^T @ E^T -> relu -> H_cc  [128, BC] ----
            # cc = 4g + r lives at row-group r, free column g; four chunks run
            # concurrently in the PE array via tile_position row packing.
            h_tiles = [None] * NCHUNK
            for g in range(NCHUNK // 4):
                for half in range(BC // HALF):
                    for r in range(4):
                        cc = 4 * g + r
                        if h_tiles[cc] is None:
                            h_tiles[cc] = hbuf.tile(
                                [128, BC], bf16, tag=f"h{cc}", name=f"h{cc}"
                            )
                        z_ps = zpsum.tile([128, HALF], fp32)
                        nc.tensor.matmul(
                            z_ps[:],
                            sb_zlhs[32 * r : 32 * (r + 1), g, :],
                            sb_ET[32 * r : 32 * (r + 1), half * HALF : (half + 1) * HALF],
                            start=True,
                            stop=True,
                            tile_position=(32 * r, 0),
                        )
                        dst = h_tiles[cc][:, half * HALF : (half + 1) * HALF]
                        if (cc + half) % 2 == 0:
                            nc.scalar.activation(out=dst, in_=z_ps[:], func=AF.Relu)
                        else:
                            nc.vector.tensor_scalar_max(dst, z_ps[:], 0.0)

            # ---- output: out[bq] = (sum_cc H_cc^T[:, bq128].T @ W2_cc + S*b2) * r ----
            o_all = outsb.tile([128, BC // 128, ODIM], fp32)
            for bq in range(BC // 128):
                o_ps = opsum.tile([128, ODIM], fp32)
                for cc in range(NCHUNK):
                    nc.tensor.matmul(
                        o_ps[:],
                        h_tiles[cc][:, bq * 128 : (bq + 1) * 128],
                        sb_w2[:, cc, :],
                        start=(cc == 0),
                        stop=False,
                    )
                # + S[b] * b2[o]
                nc.tensor.matmul(
                    o_ps[:],
                    sb_ET[:KPAD, bq * 128 : (bq + 1) * 128],
                    sb_b2r[:],
                    start=False,
                    stop=True,
                )
                nc.vector.tensor_scalar_mul(
                    o_all[:, bq, :], o_ps[:], sb_R2[:, bq : bq + 1]
                )
            nc.sync.dma_start(
                out=d_out.rearrange("(q p) o -> p q o", p=128), in_=o_all[:]
            )

    nc.finalize()
    return nc


def _build_nc_linear():
    """Program for inputs where no hidden dim's pre-activation interval
    crosses zero (true whenever edge weights make the z-interval one-sided,
    in particular for uniform A_hat): out = (exp(lat8) @ L) / S, all fp32."""
    import concourse.bacc as bacc
    import concourse.mybir as mybir
    from concourse.tile import TileContext
    from concourse._compat import get_trn_type

    fp32 = mybir.dt.float32
    AF = mybir.ActivationFunctionType

    nc = bacc.Bacc(get_trn_type() or "TRN2", target_bir_lowering=False, debug=True)

    d_latT = nc.dram_tensor("latT", [NNODE, BC], fp32, kind="ExternalInput")
    d_lat8 = nc.dram_tensor("lat8", [BC, NNODE], fp32, kind="ExternalInput")
    d_L = nc.dram_tensor("Lmat", [KPAD, ODIM], fp32, kind="ExternalInput")
    d_out = nc.dram_tensor("out", [BC, ODIM], fp32, kind="ExternalOutput")

    with TileContext(nc) as tc:
        with (
            tc.tile_pool(name="work", bufs=1) as work,
            tc.tile_pool(name="outsb", bufs=1) as outsb,
            tc.tile_pool(name="opsum", bufs=4, space="PSUM") as opsum,
        ):
            # preload the ACT exp table while DMAs land
            wm_exp = work.tile([KPAD, 1], fp32)
            nc.vector.memset(wm_exp[:], 0.0)
            nc.scalar.activation(out=wm_exp[:], in_=wm_exp[:], func=AF.Exp)

            sb_ET = work.tile([KPAD, BC], fp32)
            nc.vector.memset(sb_ET[:], 0.0)

            sb_latT = work.tile([NNODE, BC], fp32)
            nc.sync.dma_start(out=sb_latT[:], in_=d_latT[:])
            sb_lat8 = work.tile([128, BC // 128, NNODE], fp32)
            nc.sync.dma_start(
                out=sb_lat8[:], in_=d_lat8.rearrange("(t p) j -> p t j", p=128)
            )
            sb_L = work.tile([KPAD, ODIM], fp32)
            nc.sync.dma_start(out=sb_L[:], in_=d_L[:])

            nc.scalar.activation(out=sb_ET[:NNODE, :], in_=sb_latT[:], func=AF.Exp)

            sb_E2 = work.tile([128, BC // 128, NNODE], fp32)
            nc.scalar.activation(out=sb_E2[:], in_=sb_lat8[:], func=AF.Exp)
            sb_S2 = work.tile([128, BC // 128], fp32)
            nc.vector.reduce_sum(out=sb_S2[:], in_=sb_E2[:], axis=mybir.AxisListType.X)
            sb_R2 = work.tile([128, BC // 128], fp32)
            nc.vector.reciprocal(out=sb_R2[:], in_=sb_S2[:])

            o_all = outsb.tile([128, BC // 128, ODIM], fp32)
            for bq in range(BC // 128):
                o_ps = opsum.tile([128, ODIM], fp32)
                nc.tensor.matmul(
                    o_ps[:],
                    sb_ET[:, bq * 128 : (bq + 1) * 128],
                    sb_L[:],
                    start=True,
                    stop=True,
                )
                nc.vector.tensor_scalar_mul(
                    o_all[:, bq, :], o_ps[:], sb_R2[:, bq : bq + 1]
                )
            nc.sync.dma_start(
                out=d_out.rearrange("(q p) o -> p q o", p=128), in_=o_all[:]
            )

    nc.finalize()
    return nc


def _interval_classify(A_hat, d1, s1):
    """Exact interval of z[b,i,h] = d1[i,h] + 0.1*s1[h]*U[b,i]:
    U is a convex combination of A_hat[i,:], so U in [rowmin, rowmax]."""
    umin = A_hat.min(axis=1).astype(np.float64)   # [8]
    umax = A_hat.max(axis=1).astype(np.float64)
    s1d = s1.astype(np.float64)
    d1d = d1.astype(np.float64)
    t1 = 0.1 * s1d[None, :] * umin[:, None]
    t2 = 0.1 * s1d[None, :] * umax[:, None]
    zlo = d1d + np.minimum(t1, t2)                # [8, 256]
    zhi = d1d + np.maximum(t1, t2)
    margin = 1e-5 * (1.0 + np.abs(d1d) + 0.1 * np.abs(s1d)[None, :])
    pos = zlo >= margin
    neg = zhi <= -margin
    cross = ~(pos | neg)
    return zlo, zhi, pos, cross


def _host_base(node_features, edge_attr, W1, b1, W2, b2):
    nf = np.asarray(node_features, np.float32)
    ew = np.asarray(edge_attr, np.float32)[:, 0]
    W1 = np.asarray(W1, np.float32)
    b1 = np.asarray(b1, np.float32)

    # A_hat = D^-1/2 (A + I) D^-1/2, edges (i, j) for all i != j row-major
    src = np.array([i for i in range(NNODE) for j in range(NNODE) if i != j], np.int64)
    dst = np.array([j for i in range(NNODE) for j in range(NNODE) if i != j], np.int64)
    A = np.zeros((NNODE, NNODE), np.float32)
    A[dst, src] = ew
    A = A + np.eye(NNODE, dtype=np.float32)
    deg = A.sum(axis=1)
    dinv = np.where(deg > 0, deg.astype(np.float32) ** -0.5, 0.0).astype(np.float32)
    A_hat = dinv[:, None] * A * dinv[None, :]

    C1 = nf @ W1                      # [8, 256]
    D1 = A_hat @ C1                   # [8, 256]
    d1 = D1 + b1[None, :]             # [8, 256]
    s1 = W1.sum(axis=0)               # [256]
    cvec = A_hat.mean(axis=0)         # [8]
    return A_hat, d1, s1, cvec


def _host_L(A_hat, d1, s1, cvec, W2, b2, pos):
    """Fold the (exactly linear) positive dims plus b2 into L[j, o]:
    out_psum[b, o] = sum_j E[b, j] * L[j, o]   (before the 1/S scale)."""
    Ad = A_hat.astype(np.float64)
    d1d = d1.astype(np.float64)
    s1d = s1.astype(np.float64)
    cd = cvec.astype(np.float64)
    W2d = np.asarray(W2, np.float64)
    L = np.zeros((NNODE, ODIM), np.float64)
    for i in range(NNODE):
        hsel = np.nonzero(pos[i])[0]
        if hsel.size == 0:
            continue
        # coeff[h] (per j): c_i * (d1[i,h] + 0.1*s1[h]*A_hat[i,j])
        base = cd[i] * d1d[i, hsel]                    # [H]
        slope = cd[i] * 0.1 * s1d[hsel]                # [H]
        # L[j] += sum_h (base + slope*A[i,j]) * W2[h, :]
        L += np.outer(np.full(NNODE, 1.0), base @ W2d[hsel, :])
        L += np.outer(Ad[i, :], slope @ W2d[hsel, :])
    L += np.asarray(b2, np.float64)[None, :]
    Lp = np.zeros((KPAD, ODIM), np.float32)
    Lp[:NNODE] = L.astype(np.float32)
    return Lp


def _host_constants(node_features, edge_attr, W1, b1, W2, b2):
    W2 = np.asarray(W2, np.float32)
    b2 = np.asarray(b2, np.float32)
    A_hat, d1, s1, cvec = _host_base(node_features, edge_attr, W1, b1, W2, b2)

    p = np.arange(128)
    ip = p // 16                      # node index per partition
    qp = p % 16                       # h sub-index per partition

    import ml_dtypes
    bf16 = ml_dtypes.bfloat16

    # zlhs[j, cc, p] = d1[i(p), h(cc,p)] + 0.1*s1[h(cc,p)]*A_hat[i(p), j],
    # packed for 4x row-tiling: chunk cc = 4g + r at partitions 32r+j, column g
    zlhs = np.zeros((128, NCHUNK // 4, 128), np.float32)
    for cc in range(NCHUNK):
        h = cc * 16 + qp              # [128]
        g, r = cc // 4, cc % 4
        zlhs[32 * r : 32 * r + NNODE, g, :] = (
            d1[ip, h][None, :] + 0.1 * s1[h][None, :] * A_hat[ip, :].T
        )

    # w2pk[p, cc, o] = c[i(p)] * W2[h(cc,p), o]
    w2pk = np.empty((128, NCHUNK, ODIM), np.float32)
    for cc in range(NCHUNK):
        h = cc * 16 + qp
        w2pk[:, cc, :] = cvec[ip][:, None] * W2[h, :]

    b2r = np.zeros((KPAD, ODIM), np.float32)
    b2r[:NNODE, :] = b2[None, :]
    return zlhs.astype(bf16), w2pk.astype(bf16), b2r.astype(bf16)


def _run(nc, in_maps):
    global LAST_RESULTS
    from concourse.bass_utils import run_bass_kernel_spmd

    trace = bool(int(os.environ.get("GNN_TRACE", "0")))
    kwargs = {}
    if trace:
        kwargs["trace"] = True
        kwargs["trace_cores"] = [
            int(x) for x in os.environ.get("GNN_TRACE_CORES", "0").split(",")
        ]
    res = run_bass_kernel_spmd(nc, in_maps, core_ids=list(range(NCORES)), **kwargs)
    LAST_RESULTS = res
    return np.concatenate([res.results[c]["out"] for c in range(NCORES)], axis=0)


def kernel(latent_vec, node_features, edge_attr, W1, b1, W2, b2):
    lat8 = np.ascontiguousarray(np.asarray(latent_vec, np.float32)[:, :NNODE])

    A_hat, d1, s1, cvec = _host_base(node_features, edge_attr, W1, b1, W2, b2)
    zlo, zhi, pos, cross = _interval_classify(A_hat, d1, s1)

    # Ambiguous (near-zero / interval-crossing) dims: folding as linear errs
    # by at most max(0,-zlo); folding as zero errs by at most max(0,zhi).
    # Take the cheaper side per dim and bound the total output error.
    lin_err = np.maximum(0.0, -zlo)
    zero_err = np.maximum(0.0, zhi)
    fold_linear = cross & (lin_err <= zero_err)
    per_dim_err = np.where(cross, np.minimum(lin_err, zero_err), 0.0)  # [8, 256]
    W2a = np.abs(np.asarray(W2, np.float64))                            # [256, O]
    err_bound = ((np.abs(cvec)[:, None] * per_dim_err) @ W2a).max()

    if err_bound <= 5e-4:
        # Every hidden dim is (to within err_bound) linear or zero over the
        # reachable softmax range: out = (exp(lat8) @ L) / S, fp32-exact.
        Lmat = _host_L(A_hat, d1, s1, cvec, W2, b2, pos | fold_linear)
        if "nc_lin" not in _CACHE:
            _CACHE["nc_lin"] = _build_nc_linear()
        in_maps = []
        for c in range(NCORES):
            sl = lat8[c * BC : (c + 1) * BC]
            in_maps.append({
                "latT": np.ascontiguousarray(sl.T),
                "lat8": np.ascontiguousarray(sl),
                "Lmat": Lmat,
            })
        return _run(_CACHE["nc_lin"], in_maps)

    # general path: full relu machinery on all dims
    if "nc" not in _CACHE:
        _CACHE["nc"] = _build_nc()
    zlhs, w2pk, b2r = _host_constants(node_features, edge_attr, W1, b1, W2, b2)
    in_maps = []
    for c in range(NCORES):
        sl = lat8[c * BC : (c + 1) * BC]
        in_maps.append({
            "latT": np.ascontiguousarray(sl.T),
            "lat8": np.ascontiguousarray(sl),
            "zlhs": zlhs,
            "w2pk": w2pk,
            "b2r": b2r,
        })
    return _run(_CACHE["nc"], in_maps)
